# revision 14
# baseline (speedup 1.0000x reference)
"""Trainium2 Bass kernel for nn_DeepFakeDetectionModel (dense MLP).

Model: stem LN(26)->Linear(26->1024)->ReLU, 8x [LN(1024)->Linear(1024->1024)->ReLU],
head LN(1024)->Linear(1024->1)->sigmoid.  B=65536 sharded over 8 cores (data parallel,
8192 rows/core), parameters replicated.

Per-core design (batch-major pipeline, fp16 matmuls):
- Host folds each LN's affine (gamma/beta) into the following Linear:
    W' = W * gamma[None, :],  b' = b + W @ beta
  so the device only computes plain normalization (x - mean) * rsqrt(var + eps).
- Activations live as [128 batch, 1024 feat] tiles in SBUF.  LN stats via bn_stats /
  bn_aggr along the free dim; normalize is ONE fused DVE tensor_scalar op.
- The normalized tile is transposed 128x128 at a time on the PE (fp16 transpose,
  output lands in PSUM as fp16), copied to SBUF by the scalar engine, then used as
  the matmul *stationary* operand; the weights [128 f_in, 1024 f_out] are the moving
  operand.  Output is [128 batch, 1024 feat] fp32 in PSUM -> bias add (DVE) -> ReLU
  (ACT) -> next layer.  All block weights stay resident in SBUF as fp16.
"""

import os
from contextlib import ExitStack

import ml_dtypes
import numpy as np

import concourse.bass as bass
import concourse.bacc as bacc
import concourse.mybir as mybir
import concourse.tile as tile
from concourse.masks import make_identity
from concourse.bass_utils import run_bass_kernel_spmd

EPS = 1e-5
P = 128
H = 1024
F_IN = 26
L = 8
KC = H // P  # k-chunks per 1024 contraction
NJ = H // 512  # 512-wide output slices per matmul group
N_CORES = 8
B = 65536
ROWS = B // N_CORES

DT16 = mybir.dt.float16
F32 = mybir.dt.float32
NP16 = np.float16

LAST_RESULTS = None
STATS_MODE = "bn"  # "bn" | "fused"


def _bcast_ap(ap, parts):
    """Prepend a stride-0 partition dim of size `parts` to a DRAM AP."""
    return bass.AP(tensor=ap.tensor, offset=ap.offset, ap=[[0, parts]] + list(ap.ap))


def _emit(ctx, tc, rows, head_bias, x_ap, stw_ap, wt_ap, bb_ap, hw_ap, out_ap):
    nc = tc.nc
    ntiles = rows // P
    G = 4  # tiles interleaved per group
    Sqrt = mybir.ActivationFunctionType.Sqrt
    Relu = mybir.ActivationFunctionType.Relu
    Sigmoid = mybir.ActivationFunctionType.Sigmoid
    Square = mybir.ActivationFunctionType.Square
    Copy = mybir.ActivationFunctionType.Copy
    sub = mybir.AluOpType.subtract
    mult = mybir.AluOpType.mult
    add = mybir.AluOpType.add

    const = ctx.enter_context(tc.tile_pool(name="const", bufs=1))

    ident = const.tile([P, P], DT16)
    make_identity(nc, ident)

    eps_t = const.tile([P, 1], F32)
    nc.vector.memset(eps_t, EPS)

    zero_t = const.tile([P, 1], F32)
    nc.vector.memset(zero_t, 0.0)

    hb_t = const.tile([P, 1], F32)
    nc.vector.memset(hb_t, head_bias)

    # biases, broadcast across all 128 partitions: [128, 9, 1024] fp16
    bb_t = const.tile([P, L + 1, H], DT16)
    nc.sync.dma_start(out=bb_t, in_=_bcast_ap(bb_ap, P))

    # stem weights [26, 1024] fp16
    stw_t = const.tile([F_IN, H], DT16)
    nc.sync.dma_start(out=stw_t, in_=stw_ap)

    # block weights, resident: per layer [128, 8, 1024] fp16 (chunk c = rows 128c..)
    wt_t = []
    for l in range(L):
        t = const.tile([P, KC, H], DT16, name=f"wt{l}")
        for c in range(KC):
            nc.sync.dma_start(out=t[:, c, :], in_=wt_ap[l, c * P:(c + 1) * P, :])
        wt_t.append(t)

    # head weights [128, 8, 2] fp16
    hw_t = const.tile([P, KC, 2], DT16)
    nc.sync.dma_start(out=hw_t, in_=hw_ap.rearrange("(c p) n -> p c n", p=P))

    # all input rows up front; all outputs accumulated on-chip
    x_all = const.tile([P, ntiles, F_IN], F32, name="x_all")
    nc.sync.dma_start(out=x_all, in_=x_ap.rearrange("(t p) f -> p t f", p=P))
    o_all = const.tile([P, ntiles, 1], F32, name="o_all")

    tmppool = ctx.enter_context(tc.tile_pool(name="tmp", bufs=2 * G))   # hb + z junk/z
    hpool = ctx.enter_context(tc.tile_pool(name="hp", bufs=G + 1))
    ztpool = ctx.enter_context(tc.tile_pool(name="ztp", bufs=G + 1))
    stpool = ctx.enter_context(tc.tile_pool(name="stp", bufs=2 * G))
    ppool = ctx.enter_context(tc.tile_pool(name="pp", bufs=3, space="PSUM"))
    tpool = ctx.enter_context(tc.tile_pool(name="tp", bufs=2, space="PSUM"))

    inv_h = 1.0 / H

    def stats_finish(s_t, q_t, hmean):
        """(sum, sumsq) -> (m, rv) tiles; hmean = 1/width."""
        m_t = stpool.tile([P, 1], F32, tag="m", name="m_t")
        nc.scalar.activation(out=m_t, in_=s_t, func=Copy, scale=hmean)
        msq = stpool.tile([P, 1], F32, tag="msq", name="msq")
        nc.scalar.activation(out=msq, in_=m_t, func=Square, bias=zero_t)
        v_t = stpool.tile([P, 1], F32, tag="v", name="v_t")
        nc.vector.tensor_scalar(v_t, q_t, hmean, msq, mult, sub)
        sd = stpool.tile([P, 1], F32, tag="sd", name="sd")
        nc.scalar.activation(out=sd, in_=v_t, func=Sqrt, bias=eps_t, scale=1.0)
        rv = stpool.tile([P, 1], F32, tag="rv", name="rv")
        nc.vector.reciprocal(out=rv, in_=sd)
        return m_t, rv

    def transpose_mm(z, kc, rhs_t, rhs_slices, p_out, nj):
        """PE transpose z chunks -> DMA copy to SBUF -> matmuls into p_out."""
        if kc == 1:
            zt_p = tpool.tile([F_IN, P], DT16, tag="zt", name="ztp")
            nc.tensor.transpose(zt_p, z, ident)
            zt_s = ztpool.tile([F_IN, P], DT16, tag="zts", name="zts")
            nc.scalar.copy(out=zt_s, in_=zt_p)
            chunks = [zt_s]
        else:
            zt_p = tpool.tile([P, kc, P], DT16, tag="zt", name="ztp")
            for c in range(kc):
                nc.tensor.transpose(zt_p[:, c, :], z[:, c * P:(c + 1) * P], ident)
            zt_s = ztpool.tile([P, kc, P], DT16, tag="zts", name="zts")
            nc.scalar.copy(out=zt_s, in_=zt_p)
            chunks = [zt_s[:, c, :] for c in range(kc)]
        for c in range(kc):
            for j in range(nj):
                nc.tensor.matmul(p_out[:, j * 512:(j + 1) * 512] if nj > 1 else p_out,
                                 lhsT=chunks[c], rhs=rhs_slices(c, j),
                                 start=(c == 0), stop=(c == kc - 1))

    def stem(i):
        """x tile -> LN -> stem matmul; returns psum [P, H]."""
        x_t = x_all[:, i, :]
        stats = stpool.tile([P, 1, 6], F32, tag="stats", name="stats")
        nc.vector.bn_stats(out=stats[:, 0, :], in_=x_t)
        mv = stpool.tile([P, 2], F32, tag="mv", name="mv")
        nc.vector.bn_aggr(out=mv, in_=stats)
        sd = stpool.tile([P, 1], F32, tag="sd", name="sd")
        nc.scalar.activation(out=sd, in_=mv[:, 1:2], func=Sqrt, bias=eps_t, scale=1.0)
        rv = stpool.tile([P, 1], F32, tag="rv", name="rv")
        nc.vector.reciprocal(out=rv, in_=sd)
        zx = tmppool.tile([P, F_IN], DT16, tag="tmp", name="zx")
        nc.vector.tensor_scalar(zx, x_t, mv[:, 0:1], rv, sub, mult)
        p_t = ppool.tile([P, H], F32, tag="p", name="p_t")
        transpose_mm(zx, 1, stw_t, lambda c, j: stw_t[:, j * 512:(j + 1) * 512],
                     p_t, NJ)
        return p_t

    def block_norm(p_t, l):
        """psum + bias -> relu(+sum) -> sumsq -> normalize; returns z [P,H] fp16."""
        hb = tmppool.tile([P, H], DT16, tag="tmp", name="hb")
        nc.vector.tensor_tensor(hb, p_t, bb_t[:, l, :], add)
        h_t = hpool.tile([P, H], DT16, tag="h", name="h_t")
        z = tmppool.tile([P, H], DT16, tag="tmp", name="z")
        if STATS_MODE == "fused":
            s_t = stpool.tile([P, 1], F32, tag="s", name="s_t")
            nc.scalar.activation(out=h_t, in_=hb, func=Relu, bias=zero_t,
                                 accum_out=s_t)
            q_t = stpool.tile([P, 1], F32, tag="q", name="q_t")
            nc.vector.affine_mul_reduce(out=z, accum_out=q_t, in0=h_t, in1=h_t,
                                        scale=1.0, bias=0.0)
            m_t, rv = stats_finish(s_t, q_t, inv_h)
        else:
            nc.scalar.activation(out=h_t, in_=hb, func=Relu, bias=zero_t)
            stats = stpool.tile([P, 2, 6], F32, tag="stats", name="stats")
            nc.vector.bn_stats(out=stats[:, 0, :], in_=h_t[:, 0:512])
            nc.vector.bn_stats(out=stats[:, 1, :], in_=h_t[:, 512:1024])
            mv = stpool.tile([P, 2], F32, tag="mv", name="mv")
            nc.vector.bn_aggr(out=mv, in_=stats)
            sd = stpool.tile([P, 1], F32, tag="sd", name="sd")
            nc.scalar.activation(out=sd, in_=mv[:, 1:2], func=Sqrt, bias=eps_t,
                                 scale=1.0)
            rv = stpool.tile([P, 1], F32, tag="rv", name="rv")
            nc.vector.reciprocal(out=rv, in_=sd)
            m_t = mv[:, 0:1]
        nc.vector.tensor_scalar(z, h_t, m_t, rv, sub, mult)
        return z

    for g0 in range(0, ntiles, G):
        tiles = list(range(g0, min(g0 + G, ntiles)))
        p = {i: stem(i) for i in tiles}
        for l in range(L):
            for i in tiles:
                z = block_norm(p[i], l)
                p_t = ppool.tile([P, H], F32, tag="p", name="p_t")
                transpose_mm(z, KC, wt_t[l],
                             lambda c, j, _l=l: wt_t[_l][:, c, j * 512:(j + 1) * 512],
                             p_t, NJ)
                p[i] = p_t
        for i in tiles:
            z = block_norm(p[i], L)
            o_p = ppool.tile([P, 2], F32, tag="p", name="o_p")
            transpose_mm(z, KC, hw_t, lambda c, j: hw_t[:, c, :], o_p, 1)
            nc.scalar.activation(out=o_all[:, i, :], in_=o_p[:, 0:1], func=Sigmoid,
                                 bias=hb_t, scale=1.0)

    nc.sync.dma_start(out=out_ap.rearrange("(t p) o -> p t o", p=P), in_=o_all)


def build_program(rows, head_bias):
    nc = bacc.Bacc("TRN2", target_bir_lowering=False, debug=False,
                   enable_asserts=False)
    x_ap = nc.dram_tensor("x", [rows, F_IN], F32, kind="ExternalInput").ap()
    stw_ap = nc.dram_tensor("stw", [F_IN, H], DT16, kind="ExternalInput").ap()
    wt_ap = nc.dram_tensor("wt", [L, H, H], DT16, kind="ExternalInput").ap()
    bb_ap = nc.dram_tensor("bb", [L + 1, H], DT16, kind="ExternalInput").ap()
    hw_ap = nc.dram_tensor("hw", [H, 2], DT16, kind="ExternalInput").ap()
    out_ap = nc.dram_tensor("out", [rows, 1], F32, kind="ExternalOutput").ap()
    with tile.TileContext(nc) as tc:
        with ExitStack() as ctx:
            _emit(ctx, tc, rows, head_bias,
                  x_ap, stw_ap, wt_ap, bb_ap, hw_ap, out_ap)
    nc.compile()
    return nc


def preprocess(inputs):
    """Fold LN affines into the following linears; build device-layout arrays."""
    f8 = np.float64
    st_w = np.asarray(inputs["st_w"], f8)
    st_g = np.asarray(inputs["st_gamma"], f8)
    st_be = np.asarray(inputs["st_beta"], f8)
    st_b = np.asarray(inputs["st_b"], f8)
    blk_w = np.asarray(inputs["blk_w"], f8)
    blk_g = np.asarray(inputs["blk_gamma"], f8)
    blk_be = np.asarray(inputs["blk_beta"], f8)
    blk_b = np.asarray(inputs["blk_b"], f8)
    last_w = np.asarray(inputs["last_w"], f8)
    last_g = np.asarray(inputs["last_gamma"], f8)
    last_be = np.asarray(inputs["last_beta"], f8)
    last_b = np.asarray(inputs["last_b"], f8)

    st_wp = st_w * st_g[None, :]
    st_bp = st_b + st_w @ st_be
    blk_wp = blk_w * blk_g[:, None, :]
    blk_bp = blk_b + np.einsum("lhk,lk->lh", blk_w, blk_be)
    last_wp = last_w * last_g[None, :]
    head_bias = float(last_b[0] + last_w[0] @ last_be)

    stw = np.ascontiguousarray(st_wp.T).astype(NP16)                 # [26, 1024]
    wt = np.ascontiguousarray(blk_wp.transpose(0, 2, 1)).astype(NP16)  # [8, fin, fout]
    bb = np.concatenate([st_bp[None, :], blk_bp], axis=0).astype(NP16)  # [9, 1024]
    hw = np.ascontiguousarray(np.repeat(last_wp.T, 2, axis=1)).astype(NP16)  # [1024, 2]
    return stw, wt, bb, hw, head_bias


def kernel(**inputs):
    global LAST_RESULTS
    x = np.ascontiguousarray(np.asarray(inputs["x"], dtype=np.float32))
    assert x.shape == (B, F_IN)
    stw, wt, bb, hw, head_bias = preprocess(inputs)

    nc = build_program(ROWS, head_bias)
    in_maps = []
    for c in range(N_CORES):
        in_maps.append({
            "x": np.ascontiguousarray(x[c * ROWS:(c + 1) * ROWS]),
            "stw": stw, "wt": wt, "bb": bb, "hw": hw,
        })
    res = run_bass_kernel_spmd(nc, in_maps, core_ids=list(range(N_CORES)))
    LAST_RESULTS = res
    if res.exec_time_ns is not None:
        print(f"HW exec time: {res.exec_time_ns} ns")
    out = np.concatenate([res.results[c]["out"] for c in range(N_CORES)], axis=0)
    return np.ascontiguousarray(out.astype(np.float32))


# revision 15
# speedup vs baseline: 1.2794x; 1.2794x over previous
"""Trainium2 Bass kernel for nn_DeepFakeDetectionModel (dense MLP).

Model: stem LN(26)->Linear(26->1024)->ReLU, 8x [LN(1024)->Linear(1024->1024)->ReLU],
head LN(1024)->Linear(1024->1)->sigmoid.  B=65536 sharded over 8 cores (data parallel,
8192 rows/core), parameters replicated.

Per-core design (batch-major pipeline, fp16 matmuls):
- Host folds each LN's affine (gamma/beta) into the following Linear:
    W' = W * gamma[None, :],  b' = b + W @ beta
  so the device only computes plain normalization (x - mean) * rsqrt(var + eps).
- Activations live as [128 batch, 1024 feat] tiles in SBUF.  LN stats via bn_stats /
  bn_aggr along the free dim; normalize is ONE fused DVE tensor_scalar op.
- The normalized tile is transposed 128x128 at a time on the PE (fp16 transpose,
  output lands in PSUM as fp16), copied to SBUF by the scalar engine, then used as
  the matmul *stationary* operand; the weights [128 f_in, 1024 f_out] are the moving
  operand.  Output is [128 batch, 1024 feat] fp32 in PSUM -> bias add (DVE) -> ReLU
  (ACT) -> next layer.  All block weights stay resident in SBUF as fp16.
"""

import os
from contextlib import ExitStack

import ml_dtypes
import numpy as np

import concourse.bass as bass
import concourse.bacc as bacc
import concourse.mybir as mybir
import concourse.tile as tile
from concourse.masks import make_identity
from concourse.bass_utils import run_bass_kernel_spmd

EPS = 1e-5
P = 128
H = 1024
F_IN = 26
L = 8
KC = H // P  # k-chunks per 1024 contraction
NJ = H // 512  # 512-wide output slices per matmul group
N_CORES = 8
B = 65536
ROWS = B // N_CORES

DT16 = mybir.dt.float16
F32 = mybir.dt.float32
NP16 = np.float16

LAST_RESULTS = None
STATS_MODE = "fused"  # "bn" | "fused"


def _bcast_ap(ap, parts):
    """Prepend a stride-0 partition dim of size `parts` to a DRAM AP."""
    return bass.AP(tensor=ap.tensor, offset=ap.offset, ap=[[0, parts]] + list(ap.ap))


def _emit(ctx, tc, rows, head_bias, x_ap, stw_ap, wt_ap, bb_ap, hw_ap, out_ap):
    nc = tc.nc
    ntiles = rows // P
    G = 5  # tiles interleaved per group
    Sqrt = mybir.ActivationFunctionType.Sqrt
    Relu = mybir.ActivationFunctionType.Relu
    Sigmoid = mybir.ActivationFunctionType.Sigmoid
    Square = mybir.ActivationFunctionType.Square
    Copy = mybir.ActivationFunctionType.Copy
    sub = mybir.AluOpType.subtract
    mult = mybir.AluOpType.mult
    add = mybir.AluOpType.add

    const = ctx.enter_context(tc.tile_pool(name="const", bufs=1))

    ident = const.tile([P, P], DT16)
    make_identity(nc, ident)

    eps_t = const.tile([P, 1], F32)
    nc.vector.memset(eps_t, EPS)

    zero_t = const.tile([P, 1], F32)
    nc.vector.memset(zero_t, 0.0)

    hb_t = const.tile([P, 1], F32)
    nc.vector.memset(hb_t, head_bias)

    # biases, broadcast across all 128 partitions: [128, 9, 1024] fp16
    bb_t = const.tile([P, L + 1, H], DT16)
    nc.sync.dma_start(out=bb_t, in_=_bcast_ap(bb_ap, P))

    # stem weights [26, 1024] fp16
    stw_t = const.tile([F_IN, H], DT16)
    nc.sync.dma_start(out=stw_t, in_=stw_ap)

    # block weights, resident: per layer [128, 8, 1024] fp16 (chunk c = rows 128c..)
    wt_t = []
    for l in range(L):
        t = const.tile([P, KC, H], DT16, name=f"wt{l}")
        for c in range(KC):
            nc.sync.dma_start(out=t[:, c, :], in_=wt_ap[l, c * P:(c + 1) * P, :])
        wt_t.append(t)

    # head weights [128, 8, 2] fp16
    hw_t = const.tile([P, KC, 2], DT16)
    nc.sync.dma_start(out=hw_t, in_=hw_ap.rearrange("(c p) n -> p c n", p=P))

    # all input rows up front; all outputs accumulated on-chip
    x_all = const.tile([P, ntiles, F_IN], F32, name="x_all")
    nc.sync.dma_start(out=x_all, in_=x_ap.rearrange("(t p) f -> p t f", p=P))
    o_all = const.tile([P, ntiles, 1], F32, name="o_all")

    tmppool = ctx.enter_context(tc.tile_pool(name="tmp", bufs=2 * G))   # hb + z junk/z
    hpool = ctx.enter_context(tc.tile_pool(name="hp", bufs=G + 1))
    ztpool = ctx.enter_context(tc.tile_pool(name="ztp", bufs=G + 1))
    stpool = ctx.enter_context(tc.tile_pool(name="stp", bufs=2 * G))
    ppool = ctx.enter_context(tc.tile_pool(name="pp", bufs=3, space="PSUM"))
    tpool = ctx.enter_context(tc.tile_pool(name="tp", bufs=2, space="PSUM"))

    inv_h = 1.0 / H

    def stats_finish(s_t, q_t, hmean):
        """(sum, sumsq) -> (m, rv) tiles; hmean = 1/width."""
        m_t = stpool.tile([P, 1], F32, tag="m", name="m_t")
        nc.scalar.activation(out=m_t, in_=s_t, func=Copy, scale=hmean)
        msq = stpool.tile([P, 1], F32, tag="msq", name="msq")
        nc.scalar.activation(out=msq, in_=m_t, func=Square, bias=zero_t)
        v_t = stpool.tile([P, 1], F32, tag="v", name="v_t")
        nc.vector.tensor_scalar(v_t, q_t, hmean, msq, mult, sub)
        sd = stpool.tile([P, 1], F32, tag="sd", name="sd")
        nc.scalar.activation(out=sd, in_=v_t, func=Sqrt, bias=eps_t, scale=1.0)
        rv = stpool.tile([P, 1], F32, tag="rv", name="rv")
        nc.vector.reciprocal(out=rv, in_=sd)
        return m_t, rv

    def transpose_mm(z, kc, rhs_t, rhs_slices, p_out, nj):
        """PE transpose z chunks -> DMA copy to SBUF -> matmuls into p_out."""
        if kc == 1:
            zt_p = tpool.tile([F_IN, P], DT16, tag="zt", name="ztp")
            nc.tensor.transpose(zt_p, z, ident)
            zt_s = ztpool.tile([F_IN, P], DT16, tag="zts", name="zts")
            nc.scalar.copy(out=zt_s, in_=zt_p)
            chunks = [zt_s]
        else:
            zt_p = tpool.tile([P, kc, P], DT16, tag="zt", name="ztp")
            for c in range(kc):
                nc.tensor.transpose(zt_p[:, c, :], z[:, c * P:(c + 1) * P], ident)
            zt_s = ztpool.tile([P, kc, P], DT16, tag="zts", name="zts")
            nc.scalar.copy(out=zt_s, in_=zt_p)
            chunks = [zt_s[:, c, :] for c in range(kc)]
        for c in range(kc):
            for j in range(nj):
                nc.tensor.matmul(p_out[:, j * 512:(j + 1) * 512] if nj > 1 else p_out,
                                 lhsT=chunks[c], rhs=rhs_slices(c, j),
                                 start=(c == 0), stop=(c == kc - 1))

    def stem(i):
        """x tile -> LN -> stem matmul; returns psum [P, H]."""
        x_t = x_all[:, i, :]
        stats = stpool.tile([P, 1, 6], F32, tag="stats", name="stats")
        nc.vector.bn_stats(out=stats[:, 0, :], in_=x_t)
        mv = stpool.tile([P, 2], F32, tag="mv", name="mv")
        nc.vector.bn_aggr(out=mv, in_=stats)
        sd = stpool.tile([P, 1], F32, tag="sd", name="sd")
        nc.scalar.activation(out=sd, in_=mv[:, 1:2], func=Sqrt, bias=eps_t, scale=1.0)
        rv = stpool.tile([P, 1], F32, tag="rv", name="rv")
        nc.vector.reciprocal(out=rv, in_=sd)
        zx = tmppool.tile([P, F_IN], DT16, tag="tmp", name="zx")
        nc.vector.tensor_scalar(zx, x_t, mv[:, 0:1], rv, sub, mult)
        p_t = ppool.tile([P, H], F32, tag="p", name="p_t")
        transpose_mm(zx, 1, stw_t, lambda c, j: stw_t[:, j * 512:(j + 1) * 512],
                     p_t, NJ)
        return p_t

    def block_norm(p_t, l):
        """psum + bias -> relu(+sums) -> H*var via AMR(bias=-m) -> normalize in place.

        Returns z = (h - m) * rsqrt(var + eps) as fp16 [P, H] (in the h tile).
        Heavy [P, H] ops are split in free-dim halves to shorten the serial
        chain (downstream half-consumers can start after the first half).
        """
        hb = tmppool.tile([P, H], DT16, tag="tmp", name="hb")
        h_t = hpool.tile([P, H], DT16, tag="h", name="h_t")
        s0 = stpool.tile([P, 1], F32, tag="s0", name="s0")
        s1 = stpool.tile([P, 1], F32, tag="s1", name="s1")
        for g, s_g in ((0, s0), (1, s1)):
            sl = slice(g * 512, (g + 1) * 512)
            nc.vector.tensor_tensor(hb[:, sl], p_t[:, sl], bb_t[:, l, sl], add)
            nc.scalar.activation(out=h_t[:, sl], in_=hb[:, sl], func=Relu,
                                 bias=zero_t, accum_out=s_g)
        mn = stpool.tile([P, 1], F32, tag="mn", name="mn")
        nc.vector.tensor_scalar(mn, s0, s1, -inv_h, add, mult)  # -(s0+s1)/H
        q_t = stpool.tile([P, 1], F32, tag="q", name="q_t")
        nc.vector.affine_mul_reduce(out=hb, accum_out=q_t, in0=h_t, in1=h_t,
                                    scale=1.0, bias=mn)  # q = H * var
        sd = stpool.tile([P, 1], F32, tag="sd", name="sd")
        nc.scalar.activation(out=sd, in_=q_t, func=Sqrt, bias=eps_t, scale=inv_h)
        rv = stpool.tile([P, 1], F32, tag="rv", name="rv")
        nc.vector.reciprocal(out=rv, in_=sd)
        for g in (0, 1):
            sl = slice(g * 512, (g + 1) * 512)
            nc.vector.tensor_scalar(h_t[:, sl], h_t[:, sl], mn, rv, add, mult)
        return h_t

    for g0 in range(0, ntiles, G):
        tiles = list(range(g0, min(g0 + G, ntiles)))
        p = {i: stem(i) for i in tiles}
        for l in range(L):
            for i in tiles:
                z = block_norm(p[i], l)
                p_t = ppool.tile([P, H], F32, tag="p", name="p_t")
                transpose_mm(z, KC, wt_t[l],
                             lambda c, j, _l=l: wt_t[_l][:, c, j * 512:(j + 1) * 512],
                             p_t, NJ)
                p[i] = p_t
        for i in tiles:
            z = block_norm(p[i], L)
            o_p = ppool.tile([P, 2], F32, tag="p", name="o_p")
            transpose_mm(z, KC, hw_t, lambda c, j: hw_t[:, c, :], o_p, 1)
            nc.scalar.activation(out=o_all[:, i, :], in_=o_p[:, 0:1], func=Sigmoid,
                                 bias=hb_t, scale=1.0)

    nc.sync.dma_start(out=out_ap.rearrange("(t p) o -> p t o", p=P), in_=o_all)


def build_program(rows, head_bias):
    nc = bacc.Bacc("TRN2", target_bir_lowering=False, debug=False,
                   enable_asserts=False)
    x_ap = nc.dram_tensor("x", [rows, F_IN], F32, kind="ExternalInput").ap()
    stw_ap = nc.dram_tensor("stw", [F_IN, H], DT16, kind="ExternalInput").ap()
    wt_ap = nc.dram_tensor("wt", [L, H, H], DT16, kind="ExternalInput").ap()
    bb_ap = nc.dram_tensor("bb", [L + 1, H], DT16, kind="ExternalInput").ap()
    hw_ap = nc.dram_tensor("hw", [H, 2], DT16, kind="ExternalInput").ap()
    out_ap = nc.dram_tensor("out", [rows, 1], F32, kind="ExternalOutput").ap()
    with tile.TileContext(nc) as tc:
        with ExitStack() as ctx:
            _emit(ctx, tc, rows, head_bias,
                  x_ap, stw_ap, wt_ap, bb_ap, hw_ap, out_ap)
    nc.compile()
    return nc


def preprocess(inputs):
    """Fold LN affines into the following linears; build device-layout arrays."""
    f8 = np.float64
    st_w = np.asarray(inputs["st_w"], f8)
    st_g = np.asarray(inputs["st_gamma"], f8)
    st_be = np.asarray(inputs["st_beta"], f8)
    st_b = np.asarray(inputs["st_b"], f8)
    blk_w = np.asarray(inputs["blk_w"], f8)
    blk_g = np.asarray(inputs["blk_gamma"], f8)
    blk_be = np.asarray(inputs["blk_beta"], f8)
    blk_b = np.asarray(inputs["blk_b"], f8)
    last_w = np.asarray(inputs["last_w"], f8)
    last_g = np.asarray(inputs["last_gamma"], f8)
    last_be = np.asarray(inputs["last_beta"], f8)
    last_b = np.asarray(inputs["last_b"], f8)

    st_wp = st_w * st_g[None, :]
    st_bp = st_b + st_w @ st_be
    blk_wp = blk_w * blk_g[:, None, :]
    blk_bp = blk_b + np.einsum("lhk,lk->lh", blk_w, blk_be)
    last_wp = last_w * last_g[None, :]
    head_bias = float(last_b[0] + last_w[0] @ last_be)

    stw = np.ascontiguousarray(st_wp.T).astype(NP16)                 # [26, 1024]
    wt = np.ascontiguousarray(blk_wp.transpose(0, 2, 1)).astype(NP16)  # [8, fin, fout]
    bb = np.concatenate([st_bp[None, :], blk_bp], axis=0).astype(NP16)  # [9, 1024]
    hw = np.ascontiguousarray(np.repeat(last_wp.T, 2, axis=1)).astype(NP16)  # [1024, 2]
    return stw, wt, bb, hw, head_bias


def kernel(**inputs):
    global LAST_RESULTS
    x = np.ascontiguousarray(np.asarray(inputs["x"], dtype=np.float32))
    assert x.shape == (B, F_IN)
    stw, wt, bb, hw, head_bias = preprocess(inputs)

    nc = build_program(ROWS, head_bias)
    in_maps = []
    for c in range(N_CORES):
        in_maps.append({
            "x": np.ascontiguousarray(x[c * ROWS:(c + 1) * ROWS]),
            "stw": stw, "wt": wt, "bb": bb, "hw": hw,
        })
    res = run_bass_kernel_spmd(nc, in_maps, core_ids=list(range(N_CORES)))
    LAST_RESULTS = res
    if res.exec_time_ns is not None:
        print(f"HW exec time: {res.exec_time_ns} ns")
    out = np.concatenate([res.results[c]["out"] for c in range(N_CORES)], axis=0)
    return np.ascontiguousarray(out.astype(np.float32))


# revision 16
# speedup vs baseline: 1.2911x; 1.0091x over previous
"""Trainium2 Bass kernel for nn_DeepFakeDetectionModel (dense MLP).

Model: stem LN(26)->Linear(26->1024)->ReLU, 8x [LN(1024)->Linear(1024->1024)->ReLU],
head LN(1024)->Linear(1024->1)->sigmoid.  B=65536 sharded over 8 cores (data parallel,
8192 rows/core), parameters replicated.

Per-core design (batch-major pipeline, fp16 matmuls):
- Host folds each LN's affine (gamma/beta) into the following Linear:
    W' = W * gamma[None, :],  b' = b + W @ beta
  so the device only computes plain normalization (x - mean) * rsqrt(var + eps).
- Activations live as [128 batch, 1024 feat] tiles in SBUF.  LN stats via bn_stats /
  bn_aggr along the free dim; normalize is ONE fused DVE tensor_scalar op.
- The normalized tile is transposed 128x128 at a time on the PE (fp16 transpose,
  output lands in PSUM as fp16), copied to SBUF by the scalar engine, then used as
  the matmul *stationary* operand; the weights [128 f_in, 1024 f_out] are the moving
  operand.  Output is [128 batch, 1024 feat] fp32 in PSUM -> bias add (DVE) -> ReLU
  (ACT) -> next layer.  All block weights stay resident in SBUF as fp16.
"""

import os
from contextlib import ExitStack

import ml_dtypes
import numpy as np

import concourse.bass as bass
import concourse.bacc as bacc
import concourse.mybir as mybir
import concourse.tile as tile
from concourse.masks import make_identity
from concourse.bass_utils import run_bass_kernel_spmd

EPS = 1e-5
P = 128
H = 1024
F_IN = 26
L = 8
KC = H // P  # k-chunks per 1024 contraction
NJ = H // 512  # 512-wide output slices per matmul group
N_CORES = 8
B = 65536
ROWS = B // N_CORES

DT16 = mybir.dt.float16
F32 = mybir.dt.float32
NP16 = np.float16

LAST_RESULTS = None
STATS_MODE = "fused"  # "bn" | "fused"


def _bcast_ap(ap, parts):
    """Prepend a stride-0 partition dim of size `parts` to a DRAM AP."""
    return bass.AP(tensor=ap.tensor, offset=ap.offset, ap=[[0, parts]] + list(ap.ap))


def _emit(ctx, tc, rows, head_bias, x_ap, stw_ap, wt_ap, bb_ap, hw_ap, out_ap):
    nc = tc.nc
    ntiles = rows // P
    G = 5  # tiles interleaved per group
    Sqrt = mybir.ActivationFunctionType.Sqrt
    Relu = mybir.ActivationFunctionType.Relu
    Sigmoid = mybir.ActivationFunctionType.Sigmoid
    Square = mybir.ActivationFunctionType.Square
    Copy = mybir.ActivationFunctionType.Copy
    sub = mybir.AluOpType.subtract
    mult = mybir.AluOpType.mult
    add = mybir.AluOpType.add

    const = ctx.enter_context(tc.tile_pool(name="const", bufs=1))

    ident = const.tile([P, P], DT16)
    make_identity(nc, ident)

    eps_t = const.tile([P, 1], F32)
    nc.vector.memset(eps_t, EPS)

    zero_t = const.tile([P, 1], F32)
    nc.vector.memset(zero_t, 0.0)

    hb_t = const.tile([P, 1], F32)
    nc.vector.memset(hb_t, head_bias)

    # all input rows up front (before the big weight DMAs; the first stems wait on this)
    x_all = const.tile([P, ntiles, F_IN], F32, name="x_all")
    nc.sync.dma_start(out=x_all, in_=x_ap.rearrange("(t p) f -> p t f", p=P))

    # biases, broadcast across all 128 partitions: [128, 9, 1024] fp16
    bb_t = const.tile([P, L + 1, H], DT16)
    nc.sync.dma_start(out=bb_t, in_=_bcast_ap(bb_ap, P))

    # stem weights [26, 1024] fp16
    stw_t = const.tile([F_IN, H], DT16)
    nc.sync.dma_start(out=stw_t, in_=stw_ap)

    # block weights, resident: per layer [128, 8, 1024] fp16 (chunk c = rows 128c..)
    wt_t = []
    for l in range(L):
        t = const.tile([P, KC, H], DT16, name=f"wt{l}")
        for c in range(KC):
            nc.sync.dma_start(out=t[:, c, :], in_=wt_ap[l, c * P:(c + 1) * P, :])
        wt_t.append(t)

    # head weights [128, 8, 2] fp16
    hw_t = const.tile([P, KC, 2], DT16)
    nc.sync.dma_start(out=hw_t, in_=hw_ap.rearrange("(c p) n -> p c n", p=P))

    # all outputs accumulated on-chip
    o_all = const.tile([P, ntiles, 1], F32, name="o_all")

    tmppool = ctx.enter_context(tc.tile_pool(name="tmp", bufs=2 * G))   # hb + z junk/z
    hpool = ctx.enter_context(tc.tile_pool(name="hp", bufs=G + 1))
    ztpool = ctx.enter_context(tc.tile_pool(name="ztp", bufs=G + 1))
    stpool = ctx.enter_context(tc.tile_pool(name="stp", bufs=2 * G))
    ppool = ctx.enter_context(tc.tile_pool(name="pp", bufs=3, space="PSUM"))
    tpool = ctx.enter_context(tc.tile_pool(name="tp", bufs=2, space="PSUM"))

    inv_h = 1.0 / H

    def stats_finish(s_t, q_t, hmean):
        """(sum, sumsq) -> (m, rv) tiles; hmean = 1/width."""
        m_t = stpool.tile([P, 1], F32, tag="m", name="m_t")
        nc.scalar.activation(out=m_t, in_=s_t, func=Copy, scale=hmean)
        msq = stpool.tile([P, 1], F32, tag="msq", name="msq")
        nc.scalar.activation(out=msq, in_=m_t, func=Square, bias=zero_t)
        v_t = stpool.tile([P, 1], F32, tag="v", name="v_t")
        nc.vector.tensor_scalar(v_t, q_t, hmean, msq, mult, sub)
        sd = stpool.tile([P, 1], F32, tag="sd", name="sd")
        nc.scalar.activation(out=sd, in_=v_t, func=Sqrt, bias=eps_t, scale=1.0)
        rv = stpool.tile([P, 1], F32, tag="rv", name="rv")
        nc.vector.reciprocal(out=rv, in_=sd)
        return m_t, rv

    def transpose_mm(z, kc, rhs_t, rhs_slices, p_out, nj):
        """PE transpose z chunks -> DMA copy to SBUF -> matmuls into p_out."""
        if kc == 1:
            zt_p = tpool.tile([F_IN, P], DT16, tag="zt", name="ztp")
            nc.tensor.transpose(zt_p, z, ident)
            zt_s = ztpool.tile([F_IN, P], DT16, tag="zts", name="zts")
            nc.scalar.copy(out=zt_s, in_=zt_p)
            chunks = [zt_s]
        else:
            zt_p = tpool.tile([P, kc, P], DT16, tag="zt", name="ztp")
            for c in range(kc):
                nc.tensor.transpose(zt_p[:, c, :], z[:, c * P:(c + 1) * P], ident)
            zt_s = ztpool.tile([P, kc, P], DT16, tag="zts", name="zts")
            nc.scalar.copy(out=zt_s, in_=zt_p)
            chunks = [zt_s[:, c, :] for c in range(kc)]
        for c in range(kc):
            for j in range(nj):
                nc.tensor.matmul(p_out[:, j * 512:(j + 1) * 512] if nj > 1 else p_out,
                                 lhsT=chunks[c], rhs=rhs_slices(c, j),
                                 start=(c == 0), stop=(c == kc - 1))

    def stem(i):
        """x tile -> LN -> stem matmul; returns psum [P, H]."""
        x_t = x_all[:, i, :]
        stats = stpool.tile([P, 1, 6], F32, tag="stats", name="stats")
        nc.vector.bn_stats(out=stats[:, 0, :], in_=x_t)
        mv = stpool.tile([P, 2], F32, tag="mv", name="mv")
        nc.vector.bn_aggr(out=mv, in_=stats)
        sd = stpool.tile([P, 1], F32, tag="sd", name="sd")
        nc.scalar.activation(out=sd, in_=mv[:, 1:2], func=Sqrt, bias=eps_t, scale=1.0)
        rv = stpool.tile([P, 1], F32, tag="rv", name="rv")
        nc.vector.reciprocal(out=rv, in_=sd)
        zx = tmppool.tile([P, F_IN], DT16, tag="tmp", name="zx")
        nc.vector.tensor_scalar(zx, x_t, mv[:, 0:1], rv, sub, mult)
        p_t = ppool.tile([P, H], F32, tag="p", name="p_t")
        transpose_mm(zx, 1, stw_t, lambda c, j: stw_t[:, j * 512:(j + 1) * 512],
                     p_t, NJ)
        return p_t

    def block_norm(p_t, l):
        """psum + bias -> relu(+sums) -> H*var via AMR(bias=-m) -> normalize in place.

        Returns z = (h - m) * rsqrt(var + eps) as fp16 [P, H] (in the h tile).
        Heavy [P, H] ops are split in free-dim halves to shorten the serial
        chain (downstream half-consumers can start after the first half).
        """
        hb = tmppool.tile([P, H], DT16, tag="tmp", name="hb")
        h_t = hpool.tile([P, H], DT16, tag="h", name="h_t")
        s0 = stpool.tile([P, 1], F32, tag="s0", name="s0")
        nc.vector.tensor_tensor(hb, p_t, bb_t[:, l, :], add)
        nc.scalar.activation(out=h_t, in_=hb, func=Relu, bias=zero_t,
                             accum_out=s0)
        mn = stpool.tile([P, 1], F32, tag="mn", name="mn")
        nc.vector.tensor_scalar(mn, s0, 0.0, -inv_h, add, mult)  # -s0/H
        q_t = stpool.tile([P, 1], F32, tag="q", name="q_t")
        nc.vector.affine_mul_reduce(out=hb, accum_out=q_t, in0=h_t, in1=h_t,
                                    scale=1.0, bias=mn)  # q = H * var
        sd = stpool.tile([P, 1], F32, tag="sd", name="sd")
        nc.scalar.activation(out=sd, in_=q_t, func=Sqrt, bias=eps_t, scale=inv_h)
        rv = stpool.tile([P, 1], F32, tag="rv", name="rv")
        nc.vector.reciprocal(out=rv, in_=sd)
        nc.vector.tensor_scalar(h_t, h_t, mn, rv, add, mult)
        return h_t

    for g0 in range(0, ntiles, G):
        tiles = list(range(g0, min(g0 + G, ntiles)))
        p = {i: stem(i) for i in tiles}
        for l in range(L):
            for i in tiles:
                z = block_norm(p[i], l)
                p_t = ppool.tile([P, H], F32, tag="p", name="p_t")
                transpose_mm(z, KC, wt_t[l],
                             lambda c, j, _l=l: wt_t[_l][:, c, j * 512:(j + 1) * 512],
                             p_t, NJ)
                p[i] = p_t
        for i in tiles:
            z = block_norm(p[i], L)
            o_p = ppool.tile([P, 2], F32, tag="p", name="o_p")
            transpose_mm(z, KC, hw_t, lambda c, j: hw_t[:, c, :], o_p, 1)
            nc.scalar.activation(out=o_all[:, i, :], in_=o_p[:, 0:1], func=Sigmoid,
                                 bias=hb_t, scale=1.0)

    nc.sync.dma_start(out=out_ap.rearrange("(t p) o -> p t o", p=P), in_=o_all)


def build_program(rows, head_bias):
    nc = bacc.Bacc("TRN2", target_bir_lowering=False, debug=False,
                   enable_asserts=False)
    x_ap = nc.dram_tensor("x", [rows, F_IN], F32, kind="ExternalInput").ap()
    stw_ap = nc.dram_tensor("stw", [F_IN, H], DT16, kind="ExternalInput").ap()
    wt_ap = nc.dram_tensor("wt", [L, H, H], DT16, kind="ExternalInput").ap()
    bb_ap = nc.dram_tensor("bb", [L + 1, H], DT16, kind="ExternalInput").ap()
    hw_ap = nc.dram_tensor("hw", [H, 2], DT16, kind="ExternalInput").ap()
    out_ap = nc.dram_tensor("out", [rows, 1], F32, kind="ExternalOutput").ap()
    with tile.TileContext(nc) as tc:
        with ExitStack() as ctx:
            _emit(ctx, tc, rows, head_bias,
                  x_ap, stw_ap, wt_ap, bb_ap, hw_ap, out_ap)
    nc.compile()
    return nc


def preprocess(inputs):
    """Fold LN affines into the following linears; build device-layout arrays."""
    f8 = np.float64
    st_w = np.asarray(inputs["st_w"], f8)
    st_g = np.asarray(inputs["st_gamma"], f8)
    st_be = np.asarray(inputs["st_beta"], f8)
    st_b = np.asarray(inputs["st_b"], f8)
    blk_w = np.asarray(inputs["blk_w"], f8)
    blk_g = np.asarray(inputs["blk_gamma"], f8)
    blk_be = np.asarray(inputs["blk_beta"], f8)
    blk_b = np.asarray(inputs["blk_b"], f8)
    last_w = np.asarray(inputs["last_w"], f8)
    last_g = np.asarray(inputs["last_gamma"], f8)
    last_be = np.asarray(inputs["last_beta"], f8)
    last_b = np.asarray(inputs["last_b"], f8)

    st_wp = st_w * st_g[None, :]
    st_bp = st_b + st_w @ st_be
    blk_wp = blk_w * blk_g[:, None, :]
    blk_bp = blk_b + np.einsum("lhk,lk->lh", blk_w, blk_be)
    last_wp = last_w * last_g[None, :]
    head_bias = float(last_b[0] + last_w[0] @ last_be)

    stw = np.ascontiguousarray(st_wp.T).astype(NP16)                 # [26, 1024]
    wt = np.ascontiguousarray(blk_wp.transpose(0, 2, 1)).astype(NP16)  # [8, fin, fout]
    bb = np.concatenate([st_bp[None, :], blk_bp], axis=0).astype(NP16)  # [9, 1024]
    hw = np.ascontiguousarray(np.repeat(last_wp.T, 2, axis=1)).astype(NP16)  # [1024, 2]
    return stw, wt, bb, hw, head_bias


def kernel(**inputs):
    global LAST_RESULTS
    x = np.ascontiguousarray(np.asarray(inputs["x"], dtype=np.float32))
    assert x.shape == (B, F_IN)
    stw, wt, bb, hw, head_bias = preprocess(inputs)

    nc = build_program(ROWS, head_bias)
    in_maps = []
    for c in range(N_CORES):
        in_maps.append({
            "x": np.ascontiguousarray(x[c * ROWS:(c + 1) * ROWS]),
            "stw": stw, "wt": wt, "bb": bb, "hw": hw,
        })
    res = run_bass_kernel_spmd(nc, in_maps, core_ids=list(range(N_CORES)))
    LAST_RESULTS = res
    if res.exec_time_ns is not None:
        print(f"HW exec time: {res.exec_time_ns} ns")
    out = np.concatenate([res.results[c]["out"] for c in range(N_CORES)], axis=0)
    return np.ascontiguousarray(out.astype(np.float32))
